# revision 43
# baseline (speedup 1.0000x reference)
"""Trainium2 Bass kernel for nn_AttentionLayer (B=2, S=2048, HIDDEN=3072, 32 heads,
head_dim=96, RoPE, causal, source quirk: v = k pre-RoPE; v-projection unused).

Sharding: tensor-parallel over heads — 4 heads per core on 8 cores.

Fully fused 1:1 pipeline: projection tile t (512 tokens) is immediately followed
by the attention+o_proj group that becomes ready after it (b0 groups pair with
tiles 0-3, b1 groups with tiles 4-7). Projection matmuls provide exp-independent
PE work that covers the ScalarE-bound attention stretches; o_proj of the
previous group covers each tile's RoPE dependency bubble. Scores PSUM tiles are
[128,1024] so the softmax exp runs as wide ACTIVATEs (half the instruction
overhead); the same slots host the projection accumulators (tag-shared), and
the o_proj slots host the PE-transposes used to build v.

Host sums the 8 partial outputs (the tensor-parallel all-reduce) and transposes.
Self-contained: hardcodes shapes; no reads of /root/problem/*.
"""

import math
import os
import sys

import numpy as np

sys.path.insert(0, "/opt/trn_rl_repo")

import ml_dtypes

BF16 = ml_dtypes.bfloat16

HEADS = 32
HIDDEN = 3072
HD = 96  # head dim
ROPE_THETA = 10000.0
N_CORES = 8
HPC = HEADS // N_CORES  # 4 heads per core
DPC = HPC * HD  # 384 dcols per core


# ---------------------------------------------------------------- host prep

def _rope_tables(S, T):
    """C,S tables [96, T] (T = B*S, position repeats per batch), sign-folded for
    the 'q*C + swap48(q)*S' formulation. bf16 to match the reference's cast."""
    inv_freq = 1.0 / (ROPE_THETA ** (np.arange(0, HD, 2, dtype=np.float64) / HD))
    pos = np.arange(S, dtype=np.float64)
    ang = pos[:, None] * inv_freq[None, :]  # [S, 48]
    cos = np.cos(ang).astype(np.float32)
    sin = np.sin(ang).astype(np.float32)
    nrep = T // S
    C = np.zeros((HD, T), dtype=np.float32)
    Sg = np.zeros((HD, T), dtype=np.float32)
    for r in range(nrep):
        sl = slice(r * S, (r + 1) * S)
        C[:48, sl] = cos.T
        C[48:, sl] = cos.T
        Sg[:48, sl] = -sin.T
        Sg[48:, sl] = sin.T
    return C.astype(BF16), Sg.astype(BF16)


def _masks(QT=512, KC=128):
    """Diagonal-block keep masks [QT//KC, KC, QT]: mask[j,kk,qq] = qq >= KC*j+kk."""
    nj = QT // KC
    qq = np.arange(QT)[None, None, :]
    kk = np.arange(KC)[None, :, None]
    j = np.arange(nj)[:, None, None]
    return (qq >= KC * j + kk).astype(BF16)


def host_inputs_for_core(core, x_t, w_qkv, w_o, C, Sg, masks):
    c = core
    wq = w_qkv[DPC * c: DPC * (c + 1)]                      # [384, 3072]
    wk = w_qkv[HIDDEN + DPC * c: HIDDEN + DPC * (c + 1)]    # [384, 3072]
    wqk_t = np.concatenate([wq, wk], 0).T.astype(BF16)   # [3072, 768]
    # pre-tile for contiguous SBUF loads: [p, dcol, hh, c]
    wqk_r = np.ascontiguousarray(
        wqk_t.reshape(24, 128, 6, 128).transpose(1, 2, 0, 3))
    w_o_t = w_o[:, DPC * c: DPC * (c + 1)].T.astype(BF16)  # [384, 3072]
    w_o_r = np.ascontiguousarray(w_o_t.reshape(3, 128, HIDDEN).transpose(1, 0, 2))
    sels = np.zeros((HPC, DPC // 128, 128), dtype=BF16)
    for h in range(HPC):
        for i in range(HD * h, HD * (h + 1)):
            sels[h, i // 128, i % 128] = 1
    return {
        "x_t": x_t,
        "wqk_t": wqk_r,
        "w_o_t": w_o_r,
        "cos_t": C,
        "sin_t": Sg,
        "masks": np.ascontiguousarray(masks[0, :, 0:128]),
        "sels": sels,
    }


def retile_x(x_t):
    """[3072, T] -> [T//512, 128, 24, 512] so each 512-token projection tile
    loads with one contiguous run per partition."""
    H, T = x_t.shape
    nt = T // 512
    # x_t[hh*128+p, t*512+c] -> out[t, p, hh, c]
    v = x_t.reshape(24, 128, nt, 512)
    return np.ascontiguousarray(v.transpose(2, 1, 0, 3))


# ---------------------------------------------------------------- device graph

def build_nc(S=2048, B=2, out_f32=False):
    import concourse.bass as bass
    import concourse.mybir as mybir
    import concourse.tile as tile
    from concourse import bacc
    from concourse.bass import ts, ds
    from concourse.masks import make_identity
    from contextlib import ExitStack

    dt = mybir.dt
    T = B * S                 # tokens total
    NT = T // 512             # 512-token tiles (b0: 0-3, b1: 4-7)
    HCH = HIDDEN // 128       # 24 hidden chunks
    NQT = S // 512            # q-tiles per batch
    NKC = S // 128            # k-chunks per batch
    SCALE = 1.0 / math.sqrt(HD)
    out_dt = dt.float32 if out_f32 else dt.bfloat16

    nc = bacc.Bacc("TRN2", target_bir_lowering=False, debug=False)

    # Pin every ScalarE activation (Exp, Ln, Copy, Identity) to the single
    # "natural_log_exp_and_others" table set: the default chooser alternates
    # between exp_and_others (scores exp) and natural_log_exp_and_others
    # (1/den via ln), paying two ~2.7us ACT_TABLE_LOAD swaps per group. The
    # set list keeps its length/order so set-id indexing is unchanged; the
    # shared functions are just stripped from the other sets.
    def _pinned_act_table_loads(self):
        import bass_rust as _bass_rust
        from concourse.hw_specs import get_activation_tables
        if not any(isinstance(i, mybir.InstActivation)
                   for blk in self.main_func.blocks for i in blk.instructions):
            return
        keep = {mybir.ActivationFunctionType.Exp,
                mybir.ActivationFunctionType.Ln,
                mybir.ActivationFunctionType.Copy,
                mybir.ActivationFunctionType.Identity}
        tables = [(k, v if k == "natural_log_exp_and_others" else (v - keep))
                  for k, v in get_activation_tables(self.m.arch).items()]
        _bass_rust.insert_act_table_loads(self, tables)

    import types as _types
    nc.insert_act_table_loads = _types.MethodType(_pinned_act_table_loads, nc)

    x_t = nc.dram_tensor("x_t", [T // 512, 128, HCH, 512], dt.bfloat16,
                     kind="ExternalInput").ap()
    wqk = nc.dram_tensor("wqk_t", [128, 2 * DPC // 128, HCH, 128], dt.bfloat16,
                     kind="ExternalInput").ap()
    wot = nc.dram_tensor("w_o_t", [128, DPC // 128, HIDDEN], dt.bfloat16,
                     kind="ExternalInput").ap()
    cosd = nc.dram_tensor("cos_t", [HD, T], dt.bfloat16, kind="ExternalInput").ap()
    sind = nc.dram_tensor("sin_t", [HD, T], dt.bfloat16, kind="ExternalInput").ap()
    maskd = nc.dram_tensor("masks", [128, 128], dt.bfloat16, kind="ExternalInput").ap()
    seld = nc.dram_tensor("sels", [HPC, DPC // 128, 128], dt.bfloat16,
                          kind="ExternalInput").ap()
    outd = nc.dram_tensor("out", [HIDDEN, T], out_dt, kind="ExternalOutput").ap()

    with tile.TileContext(nc) as tc, ExitStack() as stk:
        # ------- persistent tiles
        pers = stk.enter_context(tc.tile_pool(name="pers", bufs=1))
        k_h = [pers.tile([128, T], dt.bfloat16, tag=f"k{h}", name=f"k{h}")
               for h in range(HPC)]
        v1_h = [pers.tile([128, NKC * B, HD + 1], dt.bfloat16, tag=f"v{h}",
                          name=f"v{h}") for h in range(HPC)]
        w_o_sb = pers.tile([128, DPC // 128, HIDDEN], dt.bfloat16, tag="wo",
                           name="w_o_sb")
        mask_sb = pers.tile([128, 128], dt.bfloat16, tag="mk", name="mask_sb")
        ident = pers.tile([128, 128], dt.bfloat16, tag="id", name="ident")

        # ------- working pools (single fused scope)
        projw = stk.enter_context(tc.tile_pool(name="projw", bufs=1))
        xload = stk.enter_context(tc.tile_pool(name="xload", bufs=2))
        qpool = stk.enter_context(tc.tile_pool(name="qpool", bufs=2))
        cspool = stk.enter_context(tc.tile_pool(name="cspool", bufs=1))
        projsc = stk.enter_context(tc.tile_pool(name="projsc", bufs=2))
        swp = stk.enter_context(tc.tile_pool(name="swp", bufs=2))
        pbuf = stk.enter_context(tc.tile_pool(name="pbuf", bufs=5))
        nrm = stk.enter_context(tc.tile_pool(name="nrm", bufs=2))
        unp = stk.enter_context(tc.tile_pool(name="unp", bufs=4))
        atq = stk.enter_context(tc.tile_pool(name="atq", bufs=2))
        osb = stk.enter_context(tc.tile_pool(name="osb", bufs=2))
        sps = stk.enter_context(tc.tile_pool(name="sps", bufs=2, space="PSUM"))
        ops = stk.enter_context(tc.tile_pool(name="ops", bufs=2, space="PSUM"))
        ops2 = stk.enter_context(tc.tile_pool(name="ops2", bufs=2, space="PSUM"))

        def emit_xload(t, engines=None):
            x_g = []
            for g in range(3):
                xg = xload.tile([128, 8, 512], dt.bfloat16, tag=f"x{g}",
                                name=f"x{g}")
                eng = engines[g] if engines else nc.sync
                eng.dma_start(xg[:], x_t[t, :, ts(g, 8), :])
                x_g.append(xg)
            return x_g

        # x(t0)/x(t1) first on sync; weights stream on the scalar ring so the
        # per-dcol sc copies aren't queued behind bulk transfers
        qkw_c = []
        for dcol in range(2 * DPC // 128):
            qt_ = projw.tile([128, HCH, 128], dt.bfloat16, tag=f"qkw{dcol}",
                             name=f"qkw{dcol}")
            (nc.sync if dcol == 0 else nc.scalar).dma_start(qt_[:], wqk[:, dcol])
            qkw_c.append(qt_)
        x_tiles = {}
        x_tiles[0] = emit_xload(0)
        x_tiles[1] = emit_xload(1)
        make_identity(nc, ident)

        def emit_lateprefetch():
            # non-critical loads + one-time engine warm-ups, emitted inside
            # pair 0 so they don't delay the first projection's sc copies
            nc.scalar.dma_start(w_o_sb[:], wot)
            nc.scalar.dma_start(mask_sb[:], maskd)
            # exp ACT table load (~2.7us) + gpsimd tensor_tensor IRAM load (~6us)
            warm = projsc.tile([1, 16], dt.bfloat16, tag="warm", name="warm")
            nc.scalar.activation(warm[:], mask_sb[0:1, 0:16],
                                 mybir.ActivationFunctionType.Exp)
            warml = projsc.tile([1, 16], dt.bfloat16, tag="warm", name="warml")
            nc.scalar.activation(warml[:], mask_sb[0:1, 0:16],
                                 mybir.ActivationFunctionType.Ln)
            warm2 = projsc.tile([1, 16], dt.bfloat16, tag="warm", name="warm2")
            nc.gpsimd.tensor_tensor(warm2[:], mask_sb[0:1, 0:16],
                                    mask_sb[0:1, 0:16], mybir.AluOpType.mult)

        # scores contract over 96 partitions only (same PE cost — cycles follow
        # the streamed free dim), so no pad-row zeroing is needed anywhere
        for h in range(HPC):
            nc.gpsimd.memset(v1_h[h][:, :, HD:HD + 1], 1.0)

        # selector matrices for the normalization broadcast matmul
        # (host-built): sel[h, c, i] = 1 iff row 128c+i belongs to head h;
        # R_chunk = sel[:, c, :].T @ recips broadcasts each head's 1/den row
        sel_sb = pers.tile([HPC, DPC // 128, 128], dt.bfloat16, tag="sel",
                           name="sel_sb")
        nc.sync.dma_start(sel_sb[:], seld)

        # dcol block -> (k-head idx or None=q, head, dst_row0, src_row0, nrows)
        def segs(block):
            out = []
            r0, r1 = 128 * block, 128 * (block + 1)
            for side in (0, 1):  # 0 = q (transient), 1 = k (persistent)
                for h in range(HPC):
                    h0 = side * DPC + h * HD
                    lo, hi = max(r0, h0), min(r1, h0 + HD)
                    if lo < hi:
                        out.append((side, h, lo - h0, lo - r0, hi - lo))
            return out

        def emit_proj(t, x_g, q_t, dcols):
            """QK projection of tile t into k_h[:, tsl] and transient q tiles.
            Scatter DMAs ride the sync queue ahead of the x prefetch."""
            tsl = ds(t * 512, 512)
            for dcol in dcols:
                ps = sps.tile([128, 512], dt.float32, tag="sps", name="pps")
                for hh in range(HCH):
                    nc.tensor.matmul(
                        ps, lhsT=qkw_c[dcol][:, hh, :],
                        rhs=x_g[hh // 8][:, hh % 8, :],
                        start=(hh == 0), stop=(hh == HCH - 1))
                sc = projsc.tile([128, 512], dt.bfloat16, tag="sc", name="sc")
                nc.scalar.copy(sc, ps)
                for side, h, d0, s0, n in segs(dcol):
                    if side == 0:
                        nc.sync.dma_start(q_t[h][d0:d0 + n, :],
                                          sc[s0:s0 + n, :])
                    else:
                        nc.sync.dma_start(k_h[h][d0:d0 + n, tsl],
                                          sc[s0:s0 + n, :])

        def emit_vt(t):
            """v1[token, kc, d] = k_h[d, kc*128+token] (pre-RoPE), this t-slice.
            PE transposes land in the o_proj PSUM slots (tag-shared)."""
            for h in range(HPC):
                for kc in range(4 * t, 4 * t + 4):
                    tp = ops2.tile([128, 128], dt.bfloat16, tag="ops2", name="tp")
                    nc.tensor.transpose(tp[:, 0:HD], k_h[h][0:HD, ts(kc, 128)],
                                        ident[0:HD, 0:HD])
                    nc.vector.tensor_copy(v1_h[h][:, kc, 0:HD], tp[:, 0:HD])

        def emit_rope(t, q_t):
            """RoPE in place: v = v*C + swap48(v)*S, on k_h t-slice + q tiles."""
            tsl = ds(t * 512, 512)
            C_sb = cspool.tile([HD, 512], dt.bfloat16, tag="cos", name="C_sb")
            S_sb = cspool.tile([HD, 512], dt.bfloat16, tag="sin", name="S_sb")
            nc.sync.dma_start(C_sb[:], cosd[:, tsl])
            nc.sync.dma_start(S_sb[:], sind[:, tsl])
            for h in range(HPC):
                for tile_, sl in ((k_h[h], tsl), (q_t[h], ds(0, 512))):
                    sw = swp.tile([HD, 512], dt.bfloat16, tag="sw", name="sw")
                    nc.sync.dma_start(sw[0:48, :], tile_[48:HD, sl])
                    nc.sync.dma_start(sw[48:HD, :], tile_[0:48, sl])
                    nc.vector.tensor_tensor(
                        tile_[0:HD, sl], tile_[0:HD, sl], C_sb[:, :],
                        mybir.AluOpType.mult)
                    nc.vector.tensor_tensor(
                        sw[:], sw[:], S_sb[:, :], mybir.AluOpType.mult)
                    nc.vector.tensor_tensor(
                        tile_[0:HD, sl], tile_[0:HD, sl], sw[:],
                        mybir.AluOpType.add)

        def emit_oproj(g, ocs, cast_eng=None):
            """o_proj for oc blocks `ocs` of group g; output stores are merged
            4 oc-blocks per DMA (3D dst access pattern over DRAM rows).
            cast_eng picks the PSUM->SBUF evacuation engine (vector default;
            scalar for the post-attention half so the DVE stays free for the
            normalization chain)."""
            gb, gqt, gat = g
            ot = None
            for i, oc in enumerate(ocs):
                if i % 4 == 0:
                    ot = osb.tile([128, 4, 512], out_dt, tag="ot", name="ot")
                ps2 = ops2.tile([128, 512], dt.float32, tag="ops2", name="ps2")
                for ic in range(DPC // 128):
                    nc.tensor.matmul(
                        ps2, lhsT=w_o_sb[:, ic, ts(oc, 128)],
                        rhs=gat[:, ic, :],
                        start=(ic == 0), stop=(ic == DPC // 128 - 1))
                if cast_eng is None:
                    nc.vector.tensor_copy(ot[:, i % 4, :], ps2)
                else:
                    cast_eng.copy(ot[:, i % 4, :], ps2)
                if i % 4 == 3:
                    oc0 = ocs[i - 3]
                    dst = outd[ds(oc0 * 128, 512), ds(gb * S + gqt * 512, 512)]
                    # rearrange the DRAM side only — SBUF APs must keep the
                    # partition dimension first
                    nc.sync.dma_start(
                        dst.rearrange("(b p) c -> p b c", b=4), ot[:, :, :])

        def emit_attention(b, qt, q_t, head_work=None):
            """Attention for group (b, qt): 4 heads, causal, softmax without
            max-subtraction; denominator via appended ones-column in v.
            Off-diagonal score chunks are computed in pairs into [128,1024]
            PSUM tiles so exp runs as wide ACTIVATEs; the 4 diagonal chunks
            pack into two tiles ((512,384) and (256,128))."""
            at_qt = atq.tile([128, DPC // 128, 512], dt.bfloat16, tag="atq",
                             name="at_qt")
            sums4 = nrm.tile([HPC, 512], dt.bfloat16, tag="sums4", name="sums4")
            for h in range(HPC):
                nlive = 4 * (qt + 1)
                o_ps = ops.tile([128, 512], dt.float32, tag="ops", name="o_ps")
                # off-diagonal chunks, two 512-wide chunks per PSUM tile
                for pair in range(2 * qt):
                    kc0 = 2 * pair
                    s_ps = sps.tile([128, 1024], dt.float32, tag="sps",
                                    name="s_ps")
                    for j in range(2):
                        nc.tensor.matmul(
                            s_ps[:, ts(j, 512)],
                            lhsT=k_h[h][0:HD, ds(b * S + (kc0 + j) * 128, 128)],
                            rhs=q_t[h][0:HD, :], start=True, stop=True)
                    p_sb = pbuf.tile([128, 1024], dt.bfloat16, tag="p",
                                     name="p_sb")
                    nc.scalar.activation(
                        p_sb[:], s_ps[:], mybir.ActivationFunctionType.Exp,
                        scale=SCALE)
                    for j in range(2):
                        nc.tensor.matmul(
                            o_ps[0:HD + 1, :],
                            lhsT=v1_h[h][:, b * NKC + kc0 + j, :],
                            rhs=p_sb[:, ts(j, 512)],
                            start=(kc0 + j == 0), stop=False)
                # diagonal chunks: widths (512, 384) then (256, 128), packed
                for dpair in range(2):
                    ws = (512, 384) if dpair == 0 else (256, 128)
                    offs = (0, 512) if dpair == 0 else (0, 256)
                    s_ps = sps.tile([128, 1024], dt.float32, tag="sps",
                                    name="s_psd")
                    for j2 in range(2):
                        j = 2 * dpair + j2
                        kc = 4 * qt + j
                        w, off = ws[j2], offs[j2]
                        nc.tensor.matmul(
                            s_ps[:, off:off + w],
                            lhsT=k_h[h][0:HD, ds(b * S + kc * 128, 128)],
                            rhs=q_t[h][0:HD, 128 * j:], start=True, stop=True)
                    p_sb = pbuf.tile([128, 1024], dt.bfloat16, tag="p",
                                     name="p_sbd")
                    tot = offs[1] + ws[1]
                    nc.scalar.activation(
                        p_sb[:, 0:tot], s_ps[:, 0:tot],
                        mybir.ActivationFunctionType.Exp, scale=SCALE)
                    for j2 in range(2):
                        j = 2 * dpair + j2
                        kc = 4 * qt + j
                        w, off = ws[j2], offs[j2]
                        nc.gpsimd.tensor_tensor(
                            p_sb[:, off:off + 128], p_sb[:, off:off + 128],
                            mask_sb[:, :], mybir.AluOpType.mult)
                        nc.tensor.matmul(
                            o_ps[0:HD + 1, 128 * j:],
                            lhsT=v1_h[h][:, b * NKC + kc, :],
                            rhs=p_sb[:, off:off + w],
                            start=(kc == 0), stop=(kc == nlive - 1))
                # copy out unnormalized attn + sums row (releases PSUM fast)
                un = unp.tile([HD + 1, 512], dt.bfloat16, tag="un",
                              name=f"un{h}")
                nc.vector.tensor_copy(un[:, :], o_ps[0:HD + 1, :])
                nc.sync.dma_start(sums4[h:h + 1, :], un[HD:HD + 1, :])
                r0 = h * HD
                while r0 < (h + 1) * HD:
                    blk = r0 // 128
                    n = min(128 * (blk + 1), (h + 1) * HD) - r0
                    nc.sync.dma_start(
                        at_qt[r0 - 128 * blk: r0 - 128 * blk + n, blk, :],
                        un[r0 - h * HD: r0 - h * HD + n, :])
                    r0 += n
                if head_work:
                    head_work[h]()
            # one reciprocal for all 4 heads' sums; the normalization itself
            # (PE broadcast via sel matmul + in-place chunk multiply) is
            # deferred into the next pair so the reciprocal hides behind the
            # next tile's projection
            rb4 = nrm.tile([HPC, 512], dt.bfloat16, tag="rb4", name="rb4")
            lg = nrm.tile([HPC, 512], dt.float32, tag="lg", name="lg",
                          bufs=1)
            # 1/den = exp(-ln(den)) on ScalarE (idle at pair boundaries) —
            # the DVE iterative reciprocal took 3.3us on a congested queue
            nc.scalar.activation(lg[:], sums4[:],
                                 mybir.ActivationFunctionType.Ln)
            with nc.allow_low_precision(
                    reason="recip via ln/exp in bf16: scale-only error ~0.4%"):
                nc.scalar.activation(rb4[:], lg[:],
                                     mybir.ActivationFunctionType.Exp,
                                     scale=-1.0)

            def finish_norm():
                for c in range(DPC // 128):
                    r_ps = ops.tile([128, 512], dt.float32, tag="ops",
                                    name="r_ps")
                    nc.tensor.matmul(r_ps, lhsT=sel_sb[:, c, :],
                                     rhs=rb4[:, :], start=True, stop=True)
                    nc.vector.tensor_tensor(
                        at_qt[:, c, :], at_qt[:, c, :], r_ps,
                        mybir.AluOpType.mult)
            return at_qt, finish_norm

        # ------- fused pipeline: tile t pairs with its attention group.
        # o_proj of the previous group is emitted in two halves: the first
        # covers the RoPE dependency bubble, the second covers the current
        # group's normalization chain (so the tail norm is never exposed).
        q_tiles = {}

        def get_q(t):
            if t not in q_tiles:
                q_tiles[t] = [qpool.tile([128, 512], dt.bfloat16,
                                         tag=f"q{h}", name=f"qt{h}")
                              for h in range(HPC)]
            return q_tiles[t]

        pending = None
        done_dcols = {}
        for t in range(NT):
            b, qt = (0, t) if t < NQT else (1, t - NQT)
            x_g = x_tiles.pop(t)
            q_t = get_q(t)
            rest = [d for d in range(2 * DPC // 128)
                    if d not in done_dcols.get(t, ())]
            emit_proj(t, x_g, q_t, rest)
            if t == 0:
                emit_lateprefetch()
            emit_vt(t)
            # finish the previous group's normalization before the rope TTs
            # so its DVE multiplies aren't queued behind them
            if pending is not None:
                pending_fin()
            emit_rope(t, q_t)
            if pending is not None:
                emit_oproj(pending, range(0, 12))
            elif t + 1 < NT:
                # pair 0 has no pending o_proj; prefetch half of t1's
                # projection to cover the RoPE bubble instead
                done_dcols[t + 1] = [0, 1, 2]
                emit_proj(t + 1, x_tiles[t + 1], get_q(t + 1), [0, 1, 2])
            if t + 1 < NT and t + 1 not in x_tiles:
                # x prefetch after the scatters so they aren't queued behind
                # the bulk transfers on the sync ring
                x_tiles[t + 1] = emit_xload(t + 1)
            # remaining o_proj of the previous group is spread between this
            # group's heads so PE work is always available behind the per-head
            # PSUM evacuation and the final normalization chain
            ce = nc.scalar if t == NT - 1 else None
            if pending is not None:
                pg = pending
                # chunks of 4 ocs (the output-store merge granularity) after
                # heads 1-3; head 3's chunk covers the reciprocal latency
                head_work = [lambda: None] + [
                    (lambda i=i, pg=pg: emit_oproj(pg, range(12 + 4 * i,
                                                             16 + 4 * i),
                                                   cast_eng=ce))
                    for i in range(3)]
            else:
                head_work = None
            at_qt, fin = emit_attention(b, qt, q_t, head_work)
            pending = (b, qt, at_qt)
            pending_fin = fin
        pending_fin()
        emit_oproj(pending, range(0, 24), cast_eng=nc.scalar)

    return nc


# ---------------------------------------------------------------- entry point

_NC_CACHE = {}


def _get_nc(S, B):
    key = (S, B)
    if key not in _NC_CACHE:
        nc = build_nc(S=S, B=B)
        nc.finalize()
        _NC_CACHE[key] = nc
    return _NC_CACHE[key]


def kernel(x, w_qkv, w_o, _trace=False):
    from concourse import bass_utils

    B, S, _ = x.shape
    T = B * S
    xf = np.asarray(x).reshape(T, HIDDEN)
    x_t = retile_x(np.ascontiguousarray(xf.T).astype(BF16))
    w_qkv = np.asarray(w_qkv).astype(BF16)
    w_o = np.asarray(w_o).astype(BF16)
    C, Sg = _rope_tables(S, T)
    masks = _masks()

    in_maps = [host_inputs_for_core(c, x_t, w_qkv, w_o, C, Sg, masks)
               for c in range(N_CORES)]

    nc = _get_nc(S, B)
    res = bass_utils.run_bass_kernel_spmd(
        nc, in_maps, core_ids=list(range(N_CORES)), trace=_trace)

    total = np.zeros((HIDDEN, T), dtype=np.float32)
    for c in range(N_CORES):
        total += np.asarray(res.results[c]["out"], dtype=np.float32)
    out = total.T.reshape(B, S, HIDDEN).astype(BF16)
    if _trace:
        return out, res
    return out


# revision 44
# speedup vs baseline: 1.1738x; 1.1738x over previous
"""Trainium2 Bass kernel for nn_AttentionLayer (B=2, S=2048, HIDDEN=3072, 32 heads,
head_dim=96, RoPE, causal, source quirk: v = k pre-RoPE; v-projection unused).

Sharding: tensor-parallel over heads — 4 heads per core on 8 cores.

Fully fused 1:1 pipeline: projection tile t (512 tokens) is immediately followed
by the attention+o_proj group that becomes ready after it (b0 groups pair with
tiles 0-3, b1 groups with tiles 4-7). Projection matmuls provide exp-independent
PE work that covers the ScalarE-bound attention stretches; o_proj of the
previous group covers each tile's RoPE dependency bubble. Scores PSUM tiles are
[128,1024] so the softmax exp runs as wide ACTIVATEs (half the instruction
overhead); the same slots host the projection accumulators (tag-shared), and
the o_proj slots host the PE-transposes used to build v.

Host sums the 8 partial outputs (the tensor-parallel all-reduce) and transposes.
Self-contained: hardcodes shapes; no reads of /root/problem/*.
"""

import math
import os
import sys

import numpy as np

sys.path.insert(0, "/opt/trn_rl_repo")

import ml_dtypes

BF16 = ml_dtypes.bfloat16

HEADS = 32
HIDDEN = 3072
HD = 96  # head dim
ROPE_THETA = 10000.0
N_CORES = 8
HPC = HEADS // N_CORES  # 4 heads per core
DPC = HPC * HD  # 384 dcols per core


# ---------------------------------------------------------------- host prep

def _rope_tables(S, T):
    """C,S tables [96, T] (T = B*S, position repeats per batch), sign-folded for
    the 'q*C + swap48(q)*S' formulation. bf16 to match the reference's cast."""
    inv_freq = 1.0 / (ROPE_THETA ** (np.arange(0, HD, 2, dtype=np.float64) / HD))
    pos = np.arange(S, dtype=np.float64)
    ang = pos[:, None] * inv_freq[None, :]  # [S, 48]
    cos = np.cos(ang).astype(np.float32)
    sin = np.sin(ang).astype(np.float32)
    nrep = T // S
    C = np.zeros((HD, T), dtype=np.float32)
    Sg = np.zeros((HD, T), dtype=np.float32)
    for r in range(nrep):
        sl = slice(r * S, (r + 1) * S)
        C[:48, sl] = cos.T
        C[48:, sl] = cos.T
        Sg[:48, sl] = -sin.T
        Sg[48:, sl] = sin.T
    return C.astype(BF16), Sg.astype(BF16)


def _masks(QT=512, KC=128):
    """Diagonal-block keep masks [QT//KC, KC, QT]: mask[j,kk,qq] = qq >= KC*j+kk."""
    nj = QT // KC
    qq = np.arange(QT)[None, None, :]
    kk = np.arange(KC)[None, :, None]
    j = np.arange(nj)[:, None, None]
    return (qq >= KC * j + kk).astype(BF16)


def host_inputs_for_core(core, x_t, w_qkv, w_o, C, Sg, masks):
    c = core
    wq = w_qkv[DPC * c: DPC * (c + 1)]                      # [384, 3072]
    wk = w_qkv[HIDDEN + DPC * c: HIDDEN + DPC * (c + 1)]    # [384, 3072]
    wqk_t = np.concatenate([wq, wk], 0).T.astype(BF16)   # [3072, 768]
    # pre-tile for contiguous SBUF loads: [p, dcol, hh, c]
    wqk_r = np.ascontiguousarray(
        wqk_t.reshape(24, 128, 6, 128).transpose(1, 2, 0, 3))
    w_o_t = w_o[:, DPC * c: DPC * (c + 1)].T.astype(BF16)  # [384, 3072]
    w_o_r = np.ascontiguousarray(w_o_t.reshape(3, 128, HIDDEN).transpose(1, 0, 2))
    sels = np.zeros((HPC, DPC // 128, 128), dtype=BF16)
    for h in range(HPC):
        for i in range(HD * h, HD * (h + 1)):
            sels[h, i // 128, i % 128] = 1
    return {
        "x_t": x_t,
        "wqk_t": wqk_r,
        "w_o_t": w_o_r,
        "cos_t": C,
        "sin_t": Sg,
        "masks": np.ascontiguousarray(masks[0, :, 0:128]),
        "sels": sels,
    }


def retile_x(x_t):
    """[3072, T] -> [T//512, 128, 24, 512] so each 512-token projection tile
    loads with one contiguous run per partition."""
    H, T = x_t.shape
    nt = T // 512
    # x_t[hh*128+p, t*512+c] -> out[t, p, hh, c]
    v = x_t.reshape(24, 128, nt, 512)
    return np.ascontiguousarray(v.transpose(2, 1, 0, 3))


# ---------------------------------------------------------------- device graph

def build_nc(S=2048, B=2, out_f32=False):
    import concourse.bass as bass
    import concourse.mybir as mybir
    import concourse.tile as tile
    from concourse import bacc
    from concourse.bass import ts, ds
    from concourse.masks import make_identity
    from contextlib import ExitStack

    dt = mybir.dt
    T = B * S                 # tokens total
    NT = T // 512             # 512-token tiles (b0: 0-3, b1: 4-7)
    HCH = HIDDEN // 128       # 24 hidden chunks
    NQT = S // 512            # q-tiles per batch
    NKC = S // 128            # k-chunks per batch
    SCALE = 1.0 / math.sqrt(HD)
    out_dt = dt.float32 if out_f32 else dt.bfloat16

    nc = bacc.Bacc("TRN2", target_bir_lowering=False, debug=False)

    # Pin every ScalarE activation (Exp, Ln, Copy, Identity) to the single
    # "natural_log_exp_and_others" table set: the default chooser alternates
    # between exp_and_others (scores exp) and natural_log_exp_and_others
    # (1/den via ln), paying two ~2.7us ACT_TABLE_LOAD swaps per group. The
    # set list keeps its length/order so set-id indexing is unchanged; the
    # shared functions are just stripped from the other sets.
    def _pinned_act_table_loads(self):
        import bass_rust as _bass_rust
        from concourse.hw_specs import get_activation_tables
        if not any(isinstance(i, mybir.InstActivation)
                   for blk in self.main_func.blocks for i in blk.instructions):
            return
        keep = {mybir.ActivationFunctionType.Exp,
                mybir.ActivationFunctionType.Ln,
                mybir.ActivationFunctionType.Copy,
                mybir.ActivationFunctionType.Identity}
        tables = [(k, v if k == "natural_log_exp_and_others" else (v - keep))
                  for k, v in get_activation_tables(self.m.arch).items()]
        _bass_rust.insert_act_table_loads(self, tables)

    import types as _types
    nc.insert_act_table_loads = _types.MethodType(_pinned_act_table_loads, nc)

    x_t = nc.dram_tensor("x_t", [T // 512, 128, HCH, 512], dt.bfloat16,
                     kind="ExternalInput").ap()
    wqk = nc.dram_tensor("wqk_t", [128, 2 * DPC // 128, HCH, 128], dt.bfloat16,
                     kind="ExternalInput").ap()
    wot = nc.dram_tensor("w_o_t", [128, DPC // 128, HIDDEN], dt.bfloat16,
                     kind="ExternalInput").ap()
    cosd = nc.dram_tensor("cos_t", [HD, T], dt.bfloat16, kind="ExternalInput").ap()
    sind = nc.dram_tensor("sin_t", [HD, T], dt.bfloat16, kind="ExternalInput").ap()
    maskd = nc.dram_tensor("masks", [128, 128], dt.bfloat16, kind="ExternalInput").ap()
    seld = nc.dram_tensor("sels", [HPC, DPC // 128, 128], dt.bfloat16,
                          kind="ExternalInput").ap()
    outd = nc.dram_tensor("out", [HIDDEN, T], out_dt, kind="ExternalOutput").ap()

    with tile.TileContext(nc) as tc, ExitStack() as stk:
        # ------- persistent tiles
        pers = stk.enter_context(tc.tile_pool(name="pers", bufs=1))
        k_h = [pers.tile([128, T], dt.bfloat16, tag=f"k{h}", name=f"k{h}")
               for h in range(HPC)]
        v1_h = [pers.tile([128, NKC * B, HD + 1], dt.bfloat16, tag=f"v{h}",
                          name=f"v{h}") for h in range(HPC)]
        w_o_sb = pers.tile([128, DPC // 128, HIDDEN], dt.bfloat16, tag="wo",
                           name="w_o_sb")
        mask_sb = pers.tile([128, 128], dt.bfloat16, tag="mk", name="mask_sb")
        ident = pers.tile([128, 128], dt.bfloat16, tag="id", name="ident")

        # ------- working pools (single fused scope)
        projw = stk.enter_context(tc.tile_pool(name="projw", bufs=1))
        xload = stk.enter_context(tc.tile_pool(name="xload", bufs=2))
        qpool = stk.enter_context(tc.tile_pool(name="qpool", bufs=2))
        cspool = stk.enter_context(tc.tile_pool(name="cspool", bufs=2))
        projsc = stk.enter_context(tc.tile_pool(name="projsc", bufs=2))
        swp = stk.enter_context(tc.tile_pool(name="swp", bufs=2))
        pbuf = stk.enter_context(tc.tile_pool(name="pbuf", bufs=4))
        nrm = stk.enter_context(tc.tile_pool(name="nrm", bufs=2))
        unp = stk.enter_context(tc.tile_pool(name="unp", bufs=4))
        atq = stk.enter_context(tc.tile_pool(name="atq", bufs=2))
        osb = stk.enter_context(tc.tile_pool(name="osb", bufs=2))
        sps = stk.enter_context(tc.tile_pool(name="sps", bufs=2, space="PSUM"))
        ops = stk.enter_context(tc.tile_pool(name="ops", bufs=2, space="PSUM"))
        ops2 = stk.enter_context(tc.tile_pool(name="ops2", bufs=2, space="PSUM"))

        def emit_xload(t, engines=None):
            x_g = []
            for g in range(3):
                xg = xload.tile([128, 8, 512], dt.bfloat16, tag=f"x{g}",
                                name=f"x{g}")
                eng = engines[g] if engines else nc.sync
                eng.dma_start(xg[:], x_t[t, :, ts(g, 8), :])
                x_g.append(xg)
            return x_g

        # x(t0)/x(t1) first on sync; weights stream on the scalar ring so the
        # per-dcol sc copies aren't queued behind bulk transfers
        qkw_c = []
        for dcol in range(2 * DPC // 128):
            qt_ = projw.tile([128, HCH, 128], dt.bfloat16, tag=f"qkw{dcol}",
                             name=f"qkw{dcol}")
            (nc.sync if dcol == 0 else nc.scalar).dma_start(qt_[:], wqk[:, dcol])
            qkw_c.append(qt_)
        x_tiles = {}
        x_tiles[0] = emit_xload(0)
        x_tiles[1] = emit_xload(1)
        make_identity(nc, ident)

        def emit_lateprefetch():
            # non-critical loads + one-time engine warm-ups, emitted inside
            # pair 0 so they don't delay the first projection's sc copies
            nc.scalar.dma_start(w_o_sb[:], wot)
            nc.scalar.dma_start(mask_sb[:], maskd)
            # exp ACT table load (~2.7us) + gpsimd tensor_tensor IRAM load (~6us)
            warm = projsc.tile([1, 16], dt.bfloat16, tag="warm", name="warm")
            nc.scalar.activation(warm[:], mask_sb[0:1, 0:16],
                                 mybir.ActivationFunctionType.Exp)
            warml = projsc.tile([1, 16], dt.bfloat16, tag="warm", name="warml")
            nc.scalar.activation(warml[:], mask_sb[0:1, 0:16],
                                 mybir.ActivationFunctionType.Ln)
            warm2 = projsc.tile([1, 16], dt.bfloat16, tag="warm", name="warm2")
            nc.gpsimd.tensor_tensor(warm2[:], mask_sb[0:1, 0:16],
                                    mask_sb[0:1, 0:16], mybir.AluOpType.mult)

        # scores contract over 96 partitions only (same PE cost — cycles follow
        # the streamed free dim), so no pad-row zeroing is needed anywhere
        for h in range(HPC):
            nc.gpsimd.memset(v1_h[h][:, :, HD:HD + 1], 1.0)

        # selector matrices for the normalization broadcast matmul
        # (host-built): sel[h, c, i] = 1 iff row 128c+i belongs to head h;
        # R_chunk = sel[:, c, :].T @ recips broadcasts each head's 1/den row
        sel_sb = pers.tile([HPC, DPC // 128, 128], dt.bfloat16, tag="sel",
                           name="sel_sb")
        nc.sync.dma_start(sel_sb[:], seld)

        # dcol block -> (k-head idx or None=q, head, dst_row0, src_row0, nrows)
        def segs(block):
            out = []
            r0, r1 = 128 * block, 128 * (block + 1)
            for side in (0, 1):  # 0 = q (transient), 1 = k (persistent)
                for h in range(HPC):
                    h0 = side * DPC + h * HD
                    lo, hi = max(r0, h0), min(r1, h0 + HD)
                    if lo < hi:
                        out.append((side, h, lo - h0, lo - r0, hi - lo))
            return out

        def emit_proj(t, x_g, q_t, dcols):
            """QK projection of tile t into k_h[:, tsl] and transient q tiles.
            Scatter DMAs ride the sync queue ahead of the x prefetch."""
            tsl = ds(t * 512, 512)
            for dcol in dcols:
                ps = sps.tile([128, 512], dt.float32, tag="sps", name="pps")
                for hh in range(HCH):
                    nc.tensor.matmul(
                        ps, lhsT=qkw_c[dcol][:, hh, :],
                        rhs=x_g[hh // 8][:, hh % 8, :],
                        start=(hh == 0), stop=(hh == HCH - 1))
                sc = projsc.tile([128, 512], dt.bfloat16, tag="sc", name="sc")
                nc.scalar.copy(sc, ps)
                for side, h, d0, s0, n in segs(dcol):
                    if side == 0:
                        nc.sync.dma_start(q_t[h][d0:d0 + n, :],
                                          sc[s0:s0 + n, :])
                    else:
                        nc.sync.dma_start(k_h[h][d0:d0 + n, tsl],
                                          sc[s0:s0 + n, :])

        def emit_vt(t):
            """v1[token, kc, d] = k_h[d, kc*128+token] (pre-RoPE), this t-slice.
            PE transposes land in the o_proj PSUM slots (tag-shared)."""
            for h in range(HPC):
                for kc in range(4 * t, 4 * t + 4):
                    tp = ops2.tile([128, 128], dt.bfloat16, tag="ops2", name="tp")
                    nc.tensor.transpose(tp[:, 0:HD], k_h[h][0:HD, ts(kc, 128)],
                                        ident[0:HD, 0:HD])
                    nc.vector.tensor_copy(v1_h[h][:, kc, 0:HD], tp[:, 0:HD])

        def emit_rope(t, q_t):
            """RoPE in place: v = v*C + swap48(v)*S, on k_h t-slice + q tiles."""
            tsl = ds(t * 512, 512)
            C_sb = cspool.tile([HD, 512], dt.bfloat16, tag="cos", name="C_sb")
            S_sb = cspool.tile([HD, 512], dt.bfloat16, tag="sin", name="S_sb")
            nc.sync.dma_start(C_sb[:], cosd[:, tsl])
            nc.sync.dma_start(S_sb[:], sind[:, tsl])
            for h in range(HPC):
                for tile_, sl in ((k_h[h], tsl), (q_t[h], ds(0, 512))):
                    sw = swp.tile([HD, 512], dt.bfloat16, tag="sw", name="sw")
                    nc.sync.dma_start(sw[0:48, :], tile_[48:HD, sl])
                    nc.sync.dma_start(sw[48:HD, :], tile_[0:48, sl])
                    nc.vector.tensor_tensor(
                        tile_[0:HD, sl], tile_[0:HD, sl], C_sb[:, :],
                        mybir.AluOpType.mult)
                    nc.vector.tensor_tensor(
                        sw[:], sw[:], S_sb[:, :], mybir.AluOpType.mult)
                    nc.vector.tensor_tensor(
                        tile_[0:HD, sl], tile_[0:HD, sl], sw[:],
                        mybir.AluOpType.add)

        def emit_oproj(g, ocs, cast_eng=None):
            """o_proj for oc blocks `ocs` of group g; output stores are merged
            4 oc-blocks per DMA (3D dst access pattern over DRAM rows).
            cast_eng picks the PSUM->SBUF evacuation engine (vector default;
            scalar for the post-attention half so the DVE stays free for the
            normalization chain)."""
            gb, gqt, gat = g
            ot = None
            for i, oc in enumerate(ocs):
                if i % 4 == 0:
                    ot = osb.tile([128, 4, 512], out_dt, tag="ot", name="ot")
                ps2 = ops2.tile([128, 512], dt.float32, tag="ops2", name="ps2")
                for ic in range(DPC // 128):
                    nc.tensor.matmul(
                        ps2, lhsT=w_o_sb[:, ic, ts(oc, 128)],
                        rhs=gat[:, ic, :],
                        start=(ic == 0), stop=(ic == DPC // 128 - 1))
                if cast_eng is None:
                    nc.vector.tensor_copy(ot[:, i % 4, :], ps2)
                else:
                    cast_eng.copy(ot[:, i % 4, :], ps2)
                if i % 4 == 3:
                    oc0 = ocs[i - 3]
                    dst = outd[ds(oc0 * 128, 512), ds(gb * S + gqt * 512, 512)]
                    # rearrange the DRAM side only — SBUF APs must keep the
                    # partition dimension first
                    nc.sync.dma_start(
                        dst.rearrange("(b p) c -> p b c", b=4), ot[:, :, :])

        def emit_attention(b, qt, q_t, head_work=None):
            """Attention for group (b, qt): 4 heads, causal, softmax without
            max-subtraction; denominator via appended ones-column in v.
            Off-diagonal score chunks are computed in pairs into [128,1024]
            PSUM tiles so exp runs as wide ACTIVATEs; the 4 diagonal chunks
            pack into two tiles ((512,384) and (256,128))."""
            at_qt = atq.tile([128, DPC // 128, 512], dt.bfloat16, tag="atq",
                             name="at_qt")
            sums4 = nrm.tile([HPC, 512], dt.bfloat16, tag="sums4", name="sums4")
            for h in range(HPC):
                nlive = 4 * (qt + 1)
                o_ps = ops.tile([128, 512], dt.float32, tag="ops", name="o_ps")
                # off-diagonal chunks, two 512-wide chunks per PSUM tile
                for pair in range(2 * qt):
                    kc0 = 2 * pair
                    s_ps = sps.tile([128, 1024], dt.float32, tag="sps",
                                    name="s_ps")
                    for j in range(2):
                        nc.tensor.matmul(
                            s_ps[:, ts(j, 512)],
                            lhsT=k_h[h][0:HD, ds(b * S + (kc0 + j) * 128, 128)],
                            rhs=q_t[h][0:HD, :], start=True, stop=True)
                    p_sb = pbuf.tile([128, 1024], dt.bfloat16, tag="p",
                                     name="p_sb")
                    nc.scalar.activation(
                        p_sb[:], s_ps[:], mybir.ActivationFunctionType.Exp,
                        scale=SCALE)
                    for j in range(2):
                        nc.tensor.matmul(
                            o_ps[0:HD + 1, :],
                            lhsT=v1_h[h][:, b * NKC + kc0 + j, :],
                            rhs=p_sb[:, ts(j, 512)],
                            start=(kc0 + j == 0), stop=False)
                # diagonal chunks: widths (512, 384) then (256, 128), packed
                for dpair in range(2):
                    ws = (512, 384) if dpair == 0 else (256, 128)
                    offs = (0, 512) if dpair == 0 else (0, 256)
                    s_ps = sps.tile([128, 1024], dt.float32, tag="sps",
                                    name="s_psd")
                    for j2 in range(2):
                        j = 2 * dpair + j2
                        kc = 4 * qt + j
                        w, off = ws[j2], offs[j2]
                        nc.tensor.matmul(
                            s_ps[:, off:off + w],
                            lhsT=k_h[h][0:HD, ds(b * S + kc * 128, 128)],
                            rhs=q_t[h][0:HD, 128 * j:], start=True, stop=True)
                    p_sb = pbuf.tile([128, 1024], dt.bfloat16, tag="p",
                                     name="p_sbd")
                    tot = offs[1] + ws[1]
                    nc.scalar.activation(
                        p_sb[:, 0:tot], s_ps[:, 0:tot],
                        mybir.ActivationFunctionType.Exp, scale=SCALE)
                    for j2 in range(2):
                        j = 2 * dpair + j2
                        kc = 4 * qt + j
                        w, off = ws[j2], offs[j2]
                        nc.gpsimd.tensor_tensor(
                            p_sb[:, off:off + 128], p_sb[:, off:off + 128],
                            mask_sb[:, :], mybir.AluOpType.mult)
                        nc.tensor.matmul(
                            o_ps[0:HD + 1, 128 * j:],
                            lhsT=v1_h[h][:, b * NKC + kc, :],
                            rhs=p_sb[:, off:off + w],
                            start=(kc == 0), stop=(kc == nlive - 1))
                # copy out unnormalized attn + sums row (releases PSUM fast)
                un = unp.tile([HD + 1, 512], dt.bfloat16, tag="un",
                              name=f"un{h}")
                nc.vector.tensor_copy(un[:, :], o_ps[0:HD + 1, :])
                nc.sync.dma_start(sums4[h:h + 1, :], un[HD:HD + 1, :])
                r0 = h * HD
                while r0 < (h + 1) * HD:
                    blk = r0 // 128
                    n = min(128 * (blk + 1), (h + 1) * HD) - r0
                    nc.sync.dma_start(
                        at_qt[r0 - 128 * blk: r0 - 128 * blk + n, blk, :],
                        un[r0 - h * HD: r0 - h * HD + n, :])
                    r0 += n
                if head_work:
                    head_work[h]()
            # one reciprocal for all 4 heads' sums; the normalization itself
            # (PE broadcast via sel matmul + in-place chunk multiply) is
            # deferred into the next pair so the reciprocal hides behind the
            # next tile's projection
            rb4 = nrm.tile([HPC, 512], dt.bfloat16, tag="rb4", name="rb4")
            lg = nrm.tile([HPC, 512], dt.float32, tag="lg", name="lg",
                          bufs=1)
            # 1/den = exp(-ln(den)) on ScalarE (idle at pair boundaries) —
            # the DVE iterative reciprocal took 3.3us on a congested queue
            nc.scalar.activation(lg[:], sums4[:],
                                 mybir.ActivationFunctionType.Ln)
            with nc.allow_low_precision(
                    reason="recip via ln/exp in bf16: scale-only error ~0.4%"):
                nc.scalar.activation(rb4[:], lg[:],
                                     mybir.ActivationFunctionType.Exp,
                                     scale=-1.0)

            def finish_norm():
                for c in range(DPC // 128):
                    r_ps = ops.tile([128, 512], dt.float32, tag="ops",
                                    name="r_ps")
                    nc.tensor.matmul(r_ps, lhsT=sel_sb[:, c, :],
                                     rhs=rb4[:, :], start=True, stop=True)
                    nc.vector.tensor_tensor(
                        at_qt[:, c, :], at_qt[:, c, :], r_ps,
                        mybir.AluOpType.mult)
            return at_qt, finish_norm

        # ------- fused pipeline: tile t pairs with its attention group.
        # o_proj of the previous group is emitted in two halves: the first
        # covers the RoPE dependency bubble, the second covers the current
        # group's normalization chain (so the tail norm is never exposed).
        q_tiles = {}

        def get_q(t):
            if t not in q_tiles:
                q_tiles[t] = [qpool.tile([128, 512], dt.bfloat16,
                                         tag=f"q{h}", name=f"qt{h}")
                              for h in range(HPC)]
            return q_tiles[t]

        pending = None
        done_dcols = {}
        for t in range(NT):
            b, qt = (0, t) if t < NQT else (1, t - NQT)
            x_g = x_tiles.pop(t)
            q_t = get_q(t)
            rest = [d for d in range(2 * DPC // 128)
                    if d not in done_dcols.get(t, ())]
            emit_proj(t, x_g, q_t, rest)
            if t == 0:
                emit_lateprefetch()
            emit_vt(t)
            # finish the previous group's normalization before the rope TTs
            # so its DVE multiplies aren't queued behind them
            if pending is not None:
                pending_fin()
            emit_rope(t, q_t)
            if pending is not None:
                emit_oproj(pending, range(0, 12))
            elif t + 1 < NT:
                # pair 0 has no pending o_proj; prefetch half of t1's
                # projection to cover the RoPE bubble instead
                done_dcols[t + 1] = [0, 1, 2]
                emit_proj(t + 1, x_tiles[t + 1], get_q(t + 1), [0, 1, 2])
            if t + 1 < NT and t + 1 not in x_tiles:
                # x prefetch after the scatters so they aren't queued behind
                # the bulk transfers on the sync ring
                x_tiles[t + 1] = emit_xload(t + 1)
            # remaining o_proj of the previous group is spread between this
            # group's heads so PE work is always available behind the per-head
            # PSUM evacuation and the final normalization chain
            ce = nc.scalar if t == NT - 1 else None
            if pending is not None:
                pg = pending
                # chunks of 4 ocs (the output-store merge granularity) after
                # heads 1-3; head 3's chunk covers the reciprocal latency
                head_work = [lambda: None] + [
                    (lambda i=i, pg=pg: emit_oproj(pg, range(12 + 4 * i,
                                                             16 + 4 * i),
                                                   cast_eng=ce))
                    for i in range(3)]
            else:
                head_work = None
            at_qt, fin = emit_attention(b, qt, q_t, head_work)
            pending = (b, qt, at_qt)
            pending_fin = fin
        pending_fin()
        emit_oproj(pending, range(0, 24), cast_eng=nc.scalar)

    return nc


# ---------------------------------------------------------------- entry point

_NC_CACHE = {}


def _get_nc(S, B):
    key = (S, B)
    if key not in _NC_CACHE:
        nc = build_nc(S=S, B=B)
        nc.finalize()
        _NC_CACHE[key] = nc
    return _NC_CACHE[key]


def kernel(x, w_qkv, w_o, _trace=False):
    from concourse import bass_utils

    B, S, _ = x.shape
    T = B * S
    xf = np.asarray(x).reshape(T, HIDDEN)
    x_t = retile_x(np.ascontiguousarray(xf.T).astype(BF16))
    w_qkv = np.asarray(w_qkv).astype(BF16)
    w_o = np.asarray(w_o).astype(BF16)
    C, Sg = _rope_tables(S, T)
    masks = _masks()

    in_maps = [host_inputs_for_core(c, x_t, w_qkv, w_o, C, Sg, masks)
               for c in range(N_CORES)]

    nc = _get_nc(S, B)
    res = bass_utils.run_bass_kernel_spmd(
        nc, in_maps, core_ids=list(range(N_CORES)), trace=_trace)

    total = np.zeros((HIDDEN, T), dtype=np.float32)
    for c in range(N_CORES):
        total += np.asarray(res.results[c]["out"], dtype=np.float32)
    out = total.T.reshape(B, S, HIDDEN).astype(BF16)
    if _trace:
        return out, res
    return out
